# revision 4
# baseline (speedup 1.0000x reference)
"""Multi-head attention (B=64, N=577, E=768, H=8) on 8 Trainium2 NeuronCores.

Sharding: data-parallel over batch — each core gets 8 batches, full weights.

Per-core dataflow (per batch):
  x [577,768] --load--> SBUF, PE-transpose -> xT [768(6x128), 577]
  QT = Wq.T @ xT  (head-packed [96, 8*577]), KT likewise        (PE, f32r)
  V  = xT.T @ Wv + bv, scaled by 1/sqrt(D), ones-column per head (-> rowsums)
  per head h:
    eT[k,q] = KT_h.T @ QT_h   (k on partitions, 5 k-tiles)      (PE)
    expE = exp(eT)            (no max-subtraction; |e| <~ 25)   (ACT)
    aT[d,q] (+rowsum row) = sum_k V_h[k,d+1] * expE[k,q]        (PE, accum)
    attn[q,d] = transpose(aT) / rowsum                          (PE + DVE)
  out[b] assembled in SBUF [128,5,768], DMA'd back.

softmax(e)*scaling @ (x Wv + bv) == (exp(e) @ (s*(x Wv + bv))) / rowsum(exp(e)),
so the host passes Wv*s, bv*s and the kernel never multiplies by s.

All matmuls run in float32r (FP22-truncated fp32, full PE rate). The BIR
verifier requires f32r matmul operands to be produced as f32r, so the SBUF
tensors feeding matmuls (weights, xT, QT/KT, V, expE) are float32r-typed
end-to-end; PSUM accumulation stays fp32.
"""

import numpy as np
from contextlib import ExitStack

import concourse.bass as bass
import concourse.bacc as bacc
import concourse.tile as tile
from concourse import mybir, masks
from concourse.bass_utils import run_bass_kernel_spmd

F32 = mybir.dt.float32
F32R = mybir.dt.float32r
Exp = mybir.ActivationFunctionType.Exp

B, N, E, H = 64, 577, 768, 8
D = E // H  # 96
NCORES = 8
NB = B // NCORES  # batches per core
NP = 578  # N padded to even (fp32r matmul moving/dst counts must be even)
NT = (N + 127) // 128  # 5 row tiles
ROWS = [min(128, N - t * 128) for t in range(NT)]  # [128,128,128,128,65]
EC = E // 128  # 6 contraction chunks
QSPLITS = [(0, 512), (512, NP)]  # psum-bank column splits of the padded free dim
DA = D + 1  # V columns per head incl. ones column


def build(nb=NB):
    nc = bacc.Bacc("TRN2", num_devices=NCORES)
    x_d = nc.declare_dram_parameter("x", [nb, N, E], F32, isOutput=False).ap()
    wq_d = nc.declare_dram_parameter("wq", [E, E], F32R, isOutput=False).ap()
    wk_d = nc.declare_dram_parameter("wk", [E, E], F32R, isOutput=False).ap()
    wv_d = nc.declare_dram_parameter("wv", [E, E], F32R, isOutput=False).ap()
    bq_d = nc.declare_dram_parameter("bq", [E], F32, isOutput=False).ap()
    bk_d = nc.declare_dram_parameter("bk", [E], F32, isOutput=False).ap()
    bv_d = nc.declare_dram_parameter("bv", [E], F32, isOutput=False).ap()
    o_d = nc.declare_dram_parameter("out", [nb, N, E], F32, isOutput=True).ap()

    with tile.TileContext(nc) as tc:
        with ExitStack() as ctx:
            cpool = ctx.enter_context(tc.tile_pool(name="cpool", bufs=1))
            xnat_pool = ctx.enter_context(tc.tile_pool(name="xnat_pool", bufs=3))
            xt_pool = ctx.enter_context(tc.tile_pool(name="xt_pool", bufs=1))
            qt_pool = ctx.enter_context(tc.tile_pool(name="qt_pool", bufs=1))
            v_pool = ctx.enter_context(tc.tile_pool(name="v_pool", bufs=1))
            ee_pool = ctx.enter_context(tc.tile_pool(name="ee_pool", bufs=11))
            at_pool = ctx.enter_context(tc.tile_pool(name="at_pool", bufs=2))
            r_pool = ctx.enter_context(tc.tile_pool(name="r_pool", bufs=2))
            out_pool = ctx.enter_context(tc.tile_pool(name="out_pool", bufs=2))
            ps_pool = ctx.enter_context(
                tc.tile_pool(name="ps_pool", bufs=4, space="PSUM")
            )

            # ---- constants ----
            ident = cpool.tile([128, 128], F32)
            masks.make_identity(nc, ident[:, :])
            zero1 = cpool.tile([128, 1], F32)
            nc.gpsimd.memset(zero1[:, :], 0.0)
            one1 = cpool.tile([128, 1], F32)
            nc.gpsimd.memset(one1[:, :], 1.0)

            wq_sb = cpool.tile([128, EC * E], F32R)
            wk_sb = cpool.tile([128, EC * E], F32R)
            wv_sb = cpool.tile([128, EC * E], F32R)
            for w_sb, w_d in ((wq_sb, wq_d), (wk_sb, wk_d), (wv_sb, wv_d)):
                for c in range(EC):
                    nc.sync.dma_start(
                        out=w_sb[:, c * E : (c + 1) * E],
                        in_=w_d[c * 128 : (c + 1) * 128, :],
                    )
            bq_sb = cpool.tile([D, H], F32)
            bk_sb = cpool.tile([D, H], F32)
            nc.sync.dma_start(out=bq_sb[:, :], in_=bq_d.rearrange("(h d) -> d h", h=H))
            nc.sync.dma_start(out=bk_sb[:, :], in_=bk_d.rearrange("(h d) -> d h", h=H))
            bv_sb = cpool.tile([128, E], F32)
            nc.sync.dma_start(out=bv_sb[:, :], in_=bv_d.partition_broadcast(128))

            for b in range(nb):
                # ---- P0: load x, build xT [6*128, 577] as [128, 6x577] ----
                xt = xt_pool.tile([128, EC * NP], F32R, tag="xt")
                xt6 = xt.rearrange("p (c n) -> p c n", c=EC)
                nc.vector.tensor_copy(
                    xt6[:, :, N:NP], zero1[:, 0:1].unsqueeze(1).broadcast_to([128, EC, NP - N])
                )
                for t in range(NT):
                    nr = ROWS[t]
                    xnat = xnat_pool.tile([nr, E], F32, tag="xnat")
                    nc.sync.dma_start(
                        out=xnat[:, :], in_=x_d[b, t * 128 : t * 128 + nr, :]
                    )
                    for g in range(2):
                        xtp = ps_pool.tile([128, 3, nr], F32, tag="ps")
                        for j in range(3):
                            c = 3 * g + j
                            nc.tensor.matmul(
                                xtp[:, j, :],
                                xnat[:, c * 128 : (c + 1) * 128],
                                ident[:nr, :nr],
                                is_transpose=True,
                            )
                        nc.vector.tensor_copy(
                            xt6[:, 3 * g : 3 * g + 3, t * 128 : t * 128 + nr],
                            xtp[:, :, :],
                        )

                # ---- P1: projections ----
                qt = qt_pool.tile([D, H * NP], F32R, tag="qt")
                kt = qt_pool.tile([D, H * NP], F32R, tag="kt")
                for dst, w_sb, b_sb in ((qt, wq_sb, bq_sb), (kt, wk_sb, bk_sb)):
                    for h in range(H):
                        p_ps = ps_pool.tile([D, NP], F32, tag="ps")
                        for c in range(EC):
                            lhsT = w_sb[:, c * E + h * D : c * E + (h + 1) * D]
                            for q0, q1 in QSPLITS:
                                nc.tensor.matmul(
                                    p_ps[:, q0:q1],
                                    lhsT,
                                    xt6[:, c, q0:q1],
                                    start=(c == 0),
                                    stop=(c == EC - 1),
                                )
                        nc.vector.tensor_scalar_add(
                            dst[:, h * NP : (h + 1) * NP], p_ps[:, :], b_sb[:, h : h + 1]
                        )

                # V (pre-scaled by host): vaug [128, NT, H, D+1], col D = ones
                vaug = v_pool.tile([128, NT, H, DA], F32R, tag="vaug")
                nc.vector.tensor_copy(
                    vaug[:, :, :, D : D + 1],
                    one1[:, 0:1].unsqueeze(1).unsqueeze(1).broadcast_to([128, NT, H, 1]),
                )
                for t in range(NT):
                    nr = ROWS[t]
                    v_ps = ps_pool.tile([nr, E], F32, tag="ps")
                    for c in range(EC):
                        lhsT = xt6[:, c, t * 128 : t * 128 + nr]
                        for e0, e1 in ((0, 512), (512, E)):
                            nc.tensor.matmul(
                                v_ps[:, e0:e1],
                                lhsT,
                                wv_sb[:, c * E + e0 : c * E + e1],
                                start=(c == 0),
                                stop=(c == EC - 1),
                            )
                    nc.vector.tensor_add(
                        vaug[:nr, t, :, 0:D],
                        v_ps.rearrange("p (h d) -> p h d", h=H),
                        bv_sb[:nr, :].rearrange("p (h d) -> p h d", h=H),
                    )

                # ---- P2: attention, software-pipelined by one head ----
                out_sb = out_pool.tile([128, NT, E], F32, tag="out")

                def emit_energy(h):
                    ees = []
                    for j in range(NT):
                        nr = ROWS[j]
                        e_ps = ps_pool.tile([nr, NP], F32, tag="ps")
                        lhsT = kt[:, h * NP + j * 128 : h * NP + j * 128 + nr]
                        for q0, q1 in QSPLITS:
                            nc.tensor.matmul(
                                e_ps[:, q0:q1], lhsT, qt[:, h * NP + q0 : h * NP + q1]
                            )
                        ee = ee_pool.tile([nr, NP], F32R, tag="ee")
                        nc.scalar.activation(ee[:, :], e_ps[:, :], Exp)
                        ees.append(ee)
                    return ees

                def emit_av_out(h, ees):
                    av_ps = ps_pool.tile([DA, NP], F32, tag="ps")
                    for j in range(NT):
                        nr = ROWS[j]
                        for q0, q1 in QSPLITS:
                            nc.tensor.matmul(
                                av_ps[:, q0:q1],
                                vaug[:nr, j, h, :],
                                ees[j][:, q0:q1],
                                start=(j == 0),
                                stop=(j == NT - 1),
                            )
                    at_sb = at_pool.tile([DA, NP], F32, tag="at")
                    nc.vector.tensor_copy(at_sb[:, :], av_ps[:, :])
                    tr_ps = ps_pool.tile([128, NT, DA], F32, tag="ps")
                    for t in range(NT):
                        nr = ROWS[t]
                        nc.tensor.matmul(
                            tr_ps[:nr, t, :],
                            at_sb[:, t * 128 : t * 128 + nr],
                            ident[:DA, :DA],
                            is_transpose=True,
                        )
                    r = r_pool.tile([128, NT], F32, tag="r")
                    nc.vector.reciprocal(r[:, 0 : NT - 1], tr_ps[:, 0 : NT - 1, D])
                    nc.vector.reciprocal(
                        r[: ROWS[-1], NT - 1 : NT], tr_ps[: ROWS[-1], NT - 1, D : D + 1]
                    )
                    nc.vector.tensor_mul(
                        out_sb[:, 0 : NT - 1, h * D : (h + 1) * D],
                        tr_ps[:, 0 : NT - 1, 0:D],
                        r[:, 0 : NT - 1].unsqueeze(2).broadcast_to([128, NT - 1, D]),
                    )
                    nr = ROWS[-1]
                    nc.vector.tensor_mul(
                        out_sb[:nr, NT - 1, h * D : (h + 1) * D],
                        tr_ps[:nr, NT - 1, 0:D],
                        r[:nr, NT - 1 : NT].broadcast_to([nr, D]),
                    )

                prev = None
                for h in range(H):
                    ees = emit_energy(h)
                    if prev is not None:
                        emit_av_out(h - 1, prev)
                    prev = ees
                emit_av_out(H - 1, prev)

                for t in range(NT):
                    nr = ROWS[t]
                    nc.sync.dma_start(
                        out=o_d[b, t * 128 : t * 128 + nr, :], in_=out_sb[:nr, t, :]
                    )
    nc.compile()
    return nc


_CACHE = {}


def _get_nc(nb=NB):
    if nb not in _CACHE:
        _CACHE[nb] = build(nb)
    return _CACHE[nb]


def kernel(x, Wq, bq, Wk, bk, Wv, bv):
    s = float(D) ** -0.5
    x = np.ascontiguousarray(np.asarray(x, dtype=np.float32))
    in_common = {
        "wq": np.ascontiguousarray(np.asarray(Wq, dtype=np.float32)),
        "wk": np.ascontiguousarray(np.asarray(Wk, dtype=np.float32)),
        "wv": np.ascontiguousarray(np.asarray(Wv, dtype=np.float32) * s),
        "bq": np.ascontiguousarray(np.asarray(bq, dtype=np.float32)),
        "bk": np.ascontiguousarray(np.asarray(bk, dtype=np.float32)),
        "bv": np.ascontiguousarray(np.asarray(bv, dtype=np.float32) * s),
    }
    nc = _get_nc()
    in_maps = [
        {"x": x[c * NB : (c + 1) * NB], **in_common} for c in range(NCORES)
    ]
    res = run_bass_kernel_spmd(nc, in_maps, list(range(NCORES))).results
    return np.concatenate([r["out"] for r in res], axis=0)


# revision 5
# speedup vs baseline: 1.1256x; 1.1256x over previous
"""Multi-head attention (B=64, N=577, E=768, H=8) on 8 Trainium2 NeuronCores.

Sharding: data-parallel over batch — each core gets 8 batches, full weights.

Per-core dataflow (per batch):
  x [577,768] --load--> SBUF, PE-transpose -> xT [768(6x128), 577]
  QT = Wq.T @ xT  (head-packed [96, 8*577]), KT likewise        (PE, f32r)
  V  = xT.T @ Wv + bv, scaled by 1/sqrt(D), ones-column per head (-> rowsums)
  per head h:
    eT[k,q] = KT_h.T @ QT_h   (k on partitions, 5 k-tiles)      (PE)
    expE = exp(eT)            (no max-subtraction; |e| <~ 25)   (ACT)
    aT[d,q] (+rowsum row) = sum_k V_h[k,d+1] * expE[k,q]        (PE, accum)
    attn[q,d] = transpose(aT) / rowsum                          (PE + DVE)
  out[b] assembled in SBUF [128,5,768], DMA'd back.

softmax(e)*scaling @ (x Wv + bv) == (exp(e) @ (s*(x Wv + bv))) / rowsum(exp(e)),
so the host passes Wv*s, bv*s and the kernel never multiplies by s.

All matmuls run in float32r (FP22-truncated fp32, full PE rate). The BIR
verifier requires f32r matmul operands to be produced as f32r, so the SBUF
tensors feeding matmuls (weights, xT, QT/KT, V, expE) are float32r-typed
end-to-end; PSUM accumulation stays fp32.
"""

import numpy as np
from contextlib import ExitStack

import concourse.bass as bass
import concourse.bacc as bacc
import concourse.tile as tile
from concourse import mybir, masks
from concourse.bass_utils import run_bass_kernel_spmd

F32 = mybir.dt.float32
F32R = mybir.dt.float32r
F16 = mybir.dt.float16
Exp = mybir.ActivationFunctionType.Exp

B, N, E, H = 64, 577, 768, 8
D = E // H  # 96
NCORES = 8
NB = B // NCORES  # batches per core
NP = 578  # N padded to even (fp32r matmul moving/dst counts must be even)
NT = (N + 127) // 128  # 5 row tiles
ROWS = [min(128, N - t * 128) for t in range(NT)]  # [128,128,128,128,65]
EC = E // 128  # 6 contraction chunks
QSPLITS = [(0, 512), (512, NP)]  # psum-bank column splits of the padded free dim
DA = D + 1  # V columns per head incl. ones column


def build(nb=NB):
    nc = bacc.Bacc("TRN2", num_devices=NCORES)
    x_d = nc.declare_dram_parameter("x", [nb, N, E], F32, isOutput=False).ap()
    wq_d = nc.declare_dram_parameter("wq", [E, E], F16, isOutput=False).ap()
    wk_d = nc.declare_dram_parameter("wk", [E, E], F16, isOutput=False).ap()
    wv_d = nc.declare_dram_parameter("wv", [E, E], F16, isOutput=False).ap()
    bq_d = nc.declare_dram_parameter("bq", [E], F32, isOutput=False).ap()
    bk_d = nc.declare_dram_parameter("bk", [E], F32, isOutput=False).ap()
    bv_d = nc.declare_dram_parameter("bv", [E], F32, isOutput=False).ap()
    o_d = nc.declare_dram_parameter("out", [nb, N, E], F32, isOutput=True).ap()

    with tile.TileContext(nc) as tc:
        with ExitStack() as ctx:
            cpool = ctx.enter_context(tc.tile_pool(name="cpool", bufs=1))
            xnat_pool = ctx.enter_context(tc.tile_pool(name="xnat_pool", bufs=3))
            xt_pool = ctx.enter_context(tc.tile_pool(name="xt_pool", bufs=2))
            qt_pool = ctx.enter_context(tc.tile_pool(name="qt_pool", bufs=1))
            v_pool = ctx.enter_context(tc.tile_pool(name="v_pool", bufs=1))
            ee_pool = ctx.enter_context(tc.tile_pool(name="ee_pool", bufs=16))
            at_pool = ctx.enter_context(tc.tile_pool(name="at_pool", bufs=2))
            r_pool = ctx.enter_context(tc.tile_pool(name="r_pool", bufs=2))
            out_pool = ctx.enter_context(tc.tile_pool(name="out_pool", bufs=2))
            ps_pool = ctx.enter_context(
                tc.tile_pool(name="ps_pool", bufs=3, space="PSUM")
            )
            ps1_pool = ctx.enter_context(
                tc.tile_pool(name="ps1_pool", bufs=2, space="PSUM")
            )

            # ---- constants ----
            ident = cpool.tile([128, 128], F32)
            masks.make_identity(nc, ident[:, :])
            zero1 = cpool.tile([128, 1], F32)
            nc.gpsimd.memset(zero1[:, :], 0.0)
            one1 = cpool.tile([128, 1], F32)
            nc.gpsimd.memset(one1[:, :], 1.0)

            wq_sb = cpool.tile([128, EC * E], F16)
            wk_sb = cpool.tile([128, EC * E], F16)
            wv_sb = cpool.tile([128, EC * E], F16)
            for w_sb, w_d in ((wq_sb, wq_d), (wk_sb, wk_d), (wv_sb, wv_d)):
                for c in range(EC):
                    nc.sync.dma_start(
                        out=w_sb[:, c * E : (c + 1) * E],
                        in_=w_d[c * 128 : (c + 1) * 128, :],
                    )
            bq_sb = cpool.tile([D, H], F32)
            bk_sb = cpool.tile([D, H], F32)
            nc.sync.dma_start(out=bq_sb[:, :], in_=bq_d.rearrange("(h d) -> d h", h=H))
            nc.sync.dma_start(out=bk_sb[:, :], in_=bk_d.rearrange("(h d) -> d h", h=H))
            bv_sb = cpool.tile([128, E], F32)
            nc.sync.dma_start(out=bv_sb[:, :], in_=bv_d.partition_broadcast(128))

            for b in range(nb):
                # ---- P0: load x, build xT [6*128, 577] as [128, 6x577] ----
                xt = xt_pool.tile([128, EC * NP], F16, tag="xt")
                xt6 = xt.rearrange("p (c n) -> p c n", c=EC)
                nc.vector.tensor_copy(
                    xt6[:, :, N:NP], zero1[:, 0:1].unsqueeze(1).broadcast_to([128, EC, NP - N])
                )
                for t in range(NT):
                    nr = ROWS[t]
                    xnat = xnat_pool.tile([nr, E], F32, tag="xnat")
                    nc.sync.dma_start(
                        out=xnat[:, :], in_=x_d[b, t * 128 : t * 128 + nr, :]
                    )
                    for g in range(2):
                        xtp = ps1_pool.tile([128, 3, nr], F32, tag="ps1")
                        for j in range(3):
                            c = 3 * g + j
                            nc.tensor.matmul(
                                xtp[:, j, :],
                                xnat[:, c * 128 : (c + 1) * 128],
                                ident[:nr, :nr],
                                is_transpose=True,
                            )
                        nc.vector.tensor_copy(
                            xt6[:, 3 * g : 3 * g + 3, t * 128 : t * 128 + nr],
                            xtp[:, :, :],
                        )

                # ---- P1: projections ----
                qt = qt_pool.tile([D, H * NP], F16, tag="qt")
                kt = qt_pool.tile([D, H * NP], F16, tag="kt")
                for dst, w_sb, b_sb in ((qt, wq_sb, bq_sb), (kt, wk_sb, bk_sb)):
                    for h in range(H):
                        p_ps = ps_pool.tile([D, NP], F32, tag="ps")
                        for c in range(EC):
                            lhsT = w_sb[:, c * E + h * D : c * E + (h + 1) * D]
                            for q0, q1 in QSPLITS:
                                nc.tensor.matmul(
                                    p_ps[:, q0:q1],
                                    lhsT,
                                    xt6[:, c, q0:q1],
                                    start=(c == 0),
                                    stop=(c == EC - 1),
                                )
                        nc.vector.tensor_scalar_add(
                            dst[:, h * NP : (h + 1) * NP], p_ps[:, :], b_sb[:, h : h + 1]
                        )

                # V (pre-scaled by host): vaug [128, NT, H, D+1], col D = ones
                vaug = v_pool.tile([128, NT, H, DA], F32R, tag="vaug")
                nc.vector.tensor_copy(
                    vaug[:, :, :, D : D + 1],
                    one1[:, 0:1].unsqueeze(1).unsqueeze(1).broadcast_to([128, NT, H, 1]),
                )
                for t in range(NT):
                    nr = ROWS[t]
                    v_ps = ps_pool.tile([nr, E], F32, tag="ps")
                    for c in range(EC):
                        lhsT = xt6[:, c, t * 128 : t * 128 + nr]
                        for e0, e1 in ((0, 512), (512, E)):
                            nc.tensor.matmul(
                                v_ps[:, e0:e1],
                                lhsT,
                                wv_sb[:, c * E + e0 : c * E + e1],
                                start=(c == 0),
                                stop=(c == EC - 1),
                            )
                    nc.vector.tensor_add(
                        vaug[:nr, t, :, 0:D],
                        v_ps.rearrange("p (h d) -> p h d", h=H),
                        bv_sb[:nr, :].rearrange("p (h d) -> p h d", h=H),
                    )

                # ---- P2: attention, software-pipelined by one head ----
                out_sb = out_pool.tile([128, NT, E], F32, tag="out")

                def emit_energy(h):
                    ees = []
                    for j in range(NT):
                        nr = ROWS[j]
                        e_ps = ps_pool.tile([nr, NP], F32, tag="ps")
                        lhsT = kt[:, h * NP + j * 128 : h * NP + j * 128 + nr]
                        for q0, q1 in QSPLITS:
                            nc.tensor.matmul(
                                e_ps[:, q0:q1], lhsT, qt[:, h * NP + q0 : h * NP + q1]
                            )
                        ee = ee_pool.tile([nr, NP], F32R, tag="ee")
                        nc.scalar.activation(ee[:, :], e_ps[:, :], Exp)
                        ees.append(ee)
                    return ees

                def emit_av_out(h, ees):
                    av_ps = ps_pool.tile([DA, NP], F32, tag="ps")
                    for j in range(NT):
                        nr = ROWS[j]
                        for q0, q1 in QSPLITS:
                            nc.tensor.matmul(
                                av_ps[:, q0:q1],
                                vaug[:nr, j, h, :],
                                ees[j][:, q0:q1],
                                start=(j == 0),
                                stop=(j == NT - 1),
                            )
                    at_sb = at_pool.tile([DA, NP], F32, tag="at")
                    nc.vector.tensor_copy(at_sb[:, :], av_ps[:, :])
                    tr_ps = ps1_pool.tile([128, NT, DA], F32, tag="ps1")
                    for t in range(NT):
                        nr = ROWS[t]
                        nc.tensor.matmul(
                            tr_ps[:nr, t, :],
                            at_sb[:, t * 128 : t * 128 + nr],
                            ident[:DA, :DA],
                            is_transpose=True,
                        )
                    r = r_pool.tile([128, NT], F32, tag="r")
                    nc.vector.reciprocal(r[:, 0 : NT - 1], tr_ps[:, 0 : NT - 1, D])
                    nc.vector.reciprocal(
                        r[: ROWS[-1], NT - 1 : NT], tr_ps[: ROWS[-1], NT - 1, D : D + 1]
                    )
                    nc.vector.tensor_mul(
                        out_sb[:, 0 : NT - 1, h * D : (h + 1) * D],
                        tr_ps[:, 0 : NT - 1, 0:D],
                        r[:, 0 : NT - 1].unsqueeze(2).broadcast_to([128, NT - 1, D]),
                    )
                    nr = ROWS[-1]
                    nc.vector.tensor_mul(
                        out_sb[:nr, NT - 1, h * D : (h + 1) * D],
                        tr_ps[:nr, NT - 1, 0:D],
                        r[:nr, NT - 1 : NT].broadcast_to([nr, D]),
                    )

                pend = {}
                for h in range(H):
                    pend[h] = emit_energy(h)
                    if h >= 2:
                        emit_av_out(h - 2, pend.pop(h - 2))
                emit_av_out(H - 2, pend.pop(H - 2))
                emit_av_out(H - 1, pend.pop(H - 1))

                for t in range(NT):
                    nr = ROWS[t]
                    nc.sync.dma_start(
                        out=o_d[b, t * 128 : t * 128 + nr, :], in_=out_sb[:nr, t, :]
                    )
    nc.compile()
    return nc


_CACHE = {}


def _get_nc(nb=NB):
    if nb not in _CACHE:
        _CACHE[nb] = build(nb)
    return _CACHE[nb]


def kernel(x, Wq, bq, Wk, bk, Wv, bv):
    s = float(D) ** -0.5
    x = np.ascontiguousarray(np.asarray(x, dtype=np.float32))
    in_common = {
        "wq": np.ascontiguousarray(np.asarray(Wq, dtype=np.float16)),
        "wk": np.ascontiguousarray(np.asarray(Wk, dtype=np.float16)),
        "wv": np.ascontiguousarray((np.asarray(Wv, dtype=np.float32) * s).astype(np.float16)),
        "bq": np.ascontiguousarray(np.asarray(bq, dtype=np.float32)),
        "bk": np.ascontiguousarray(np.asarray(bk, dtype=np.float32)),
        "bv": np.ascontiguousarray(np.asarray(bv, dtype=np.float32) * s),
    }
    nc = _get_nc()
    in_maps = [
        {"x": x[c * NB : (c + 1) * NB], **in_common} for c in range(NCORES)
    ]
    res = run_bass_kernel_spmd(nc, in_maps, list(range(NCORES))).results
    return np.concatenate([r["out"] for r in res], axis=0)


# revision 8
# speedup vs baseline: 1.6111x; 1.4313x over previous
"""Multi-head attention (B=64, N=577, E=768, H=8) on 8 Trainium2 NeuronCores.

Sharding: data-parallel over batch — each core gets 8 batches, full weights.

Per-core dataflow (per batch):
  x [577,768] --load--> SBUF, PE-transpose -> xT [768(6x128), 577]
  QT = Wq.T @ xT  (head-packed [96, 8*578]), KT likewise        (PE, fp16)
  V  = xT.T @ Wv + bv, scaled by 1/sqrt(D), ones-column per head (-> rowsums)
  per head h:
    eT[k,q] = KT_h.T @ QT_h   (k on partitions, 5 k-tiles)      (PE, fp16)
    expE = exp(eT)            (no max-subtraction; |e| <~ 19)   (ACT, ->f32r)
    aT[d,q] (+rowsum row) = sum_k V_h[k,d+1] * expE[k,q]        (PE, f32r)
    attn[q,d] = transpose(aT) / rowsum                          (PE + DVE)
  out[b] assembled in SBUF [128,5,768], DMA'd back.

softmax(e)*scaling @ (x Wv + bv) == (exp(e) @ (s*(x Wv + bv))) / rowsum(exp(e)),
so the host passes Wv*s, bv*s (as fp16) and the kernel never multiplies by s.

Scheduling: the attention phase is rate-limited by ACT's exp (~740ns per
[128,578] tile) — PE produces an energy tile in ~540ns, so a naive ordering
stalls PE on PSUM slots every k-tile and the HAM clock gate drops the PE to
1.2 GHz. To keep PE streak-busy, batch b+1's x-load/transpose and QKV
projection matmuls are interleaved as filler work between batch b's
energy/AV matmuls (generator `p01_units` pumped inside the head loop).

fp32r matmul ISA restrictions: moving-operand and psum-dst innermost counts
must be even, dst 8B-aligned, start_partition 0 — hence the N->578 padded
free dim (pad column zeroed) and the even projection splits.
"""

import numpy as np
from contextlib import ExitStack

import concourse.bass as bass
import concourse.bacc as bacc
import concourse.tile as tile
from concourse import mybir, masks
from concourse.bass_utils import run_bass_kernel_spmd

F32 = mybir.dt.float32
F32R = mybir.dt.float32r
F16 = mybir.dt.float16
Exp = mybir.ActivationFunctionType.Exp

B, N, E, H = 64, 577, 768, 8
D = E // H  # 96
NCORES = 8
NB = B // NCORES  # batches per core
NP = 578  # N padded to even (fp32r matmul moving/dst counts must be even)
NT = (N + 127) // 128  # 5 row tiles
ROWS = [min(128, N - t * 128) for t in range(NT)]  # [128,128,128,128,65]
EC = E // 128  # 6 contraction chunks
QSPLITS = [(0, 512), (512, NP)]  # 2-bank psum column splits (energy / AV)
PSPLITS = [(0, 290), (290, NP)]  # 1-bank psum splits for Q/K projections
VSPLITS = [(0, 384), (384, E)]  # 1-bank psum splits for V projection
DA = D + 1  # V columns per head incl. ones column


def build(nb=NB):
    nc = bacc.Bacc("TRN2", num_devices=NCORES)
    x_d = nc.declare_dram_parameter("x", [nb, N, E], F32, isOutput=False).ap()
    wq_d = nc.declare_dram_parameter("wq", [E, E], F16, isOutput=False).ap()
    wk_d = nc.declare_dram_parameter("wk", [E, E], F16, isOutput=False).ap()
    wv_d = nc.declare_dram_parameter("wv", [E, E], F16, isOutput=False).ap()
    bq_d = nc.declare_dram_parameter("bq", [E], F32, isOutput=False).ap()
    bk_d = nc.declare_dram_parameter("bk", [E], F32, isOutput=False).ap()
    bv_d = nc.declare_dram_parameter("bv", [E], F32, isOutput=False).ap()
    o_d = nc.declare_dram_parameter("out", [nb, N, E], F32, isOutput=True).ap()

    with tile.TileContext(nc) as tc:
        with ExitStack() as ctx:
            cpool = ctx.enter_context(tc.tile_pool(name="cpool", bufs=1))
            xnat_pool = ctx.enter_context(tc.tile_pool(name="xnat_pool", bufs=2))
            xt_pool = ctx.enter_context(tc.tile_pool(name="xt_pool", bufs=2))
            qt_pool = ctx.enter_context(tc.tile_pool(name="qt_pool", bufs=2))
            v_pool = ctx.enter_context(tc.tile_pool(name="v_pool", bufs=2))
            ee_pool = ctx.enter_context(tc.tile_pool(name="ee_pool", bufs=15))
            at_pool = ctx.enter_context(tc.tile_pool(name="at_pool", bufs=3))
            r_pool = ctx.enter_context(tc.tile_pool(name="r_pool", bufs=2))
            out_pool = ctx.enter_context(tc.tile_pool(name="out_pool", bufs=2))
            # "big" = 2-bank tiles (energy, AV); "small" = 1-bank tiles
            # (x-transpose blocks, attn-transpose blocks, projection halves)
            psb_pool = ctx.enter_context(
                tc.tile_pool(name="psb_pool", bufs=3, space="PSUM")
            )
            pss_pool = ctx.enter_context(
                tc.tile_pool(name="pss_pool", bufs=2, space="PSUM")
            )

            # ---- constants ----
            ident = cpool.tile([128, 128], F32)
            masks.make_identity(nc, ident[:, :])
            zero1 = cpool.tile([128, 1], F32)
            nc.gpsimd.memset(zero1[:, :], 0.0)
            one1 = cpool.tile([128, 1], F32)
            nc.gpsimd.memset(one1[:, :], 1.0)

            wq_sb = cpool.tile([128, EC * E], F16)
            wk_sb = cpool.tile([128, EC * E], F16)
            wv_sb = cpool.tile([128, EC * E], F16)
            for w_sb, w_d in ((wq_sb, wq_d), (wk_sb, wk_d), (wv_sb, wv_d)):
                for c in range(EC):
                    nc.sync.dma_start(
                        out=w_sb[:, c * E : (c + 1) * E],
                        in_=w_d[c * 128 : (c + 1) * 128, :],
                    )
            bq_sb = cpool.tile([D, H], F32)
            bk_sb = cpool.tile([D, H], F32)
            nc.sync.dma_start(out=bq_sb[:, :], in_=bq_d.rearrange("(h d) -> d h", h=H))
            nc.sync.dma_start(out=bk_sb[:, :], in_=bk_d.rearrange("(h d) -> d h", h=H))
            bv_sb = cpool.tile([128, E], F32)
            nc.sync.dma_start(out=bv_sb[:, :], in_=bv_d.partition_broadcast(128))

            def p01_units(b):
                """Load/transpose x and project Q/K/V for batch b.

                Generator yielding after each small chunk of PE work
                (~3-6 matmuls) so it can be pumped as filler between the
                previous batch's attention matmuls. The final yield carries
                the produced (qt, kt, vaug) tiles.
                """
                xt = xt_pool.tile([128, EC * NP], F16, tag="xt")
                xt6 = xt.rearrange("p (c n) -> p c n", c=EC)
                nc.vector.tensor_copy(
                    xt6[:, :, N:NP],
                    zero1[:, 0:1].unsqueeze(1).broadcast_to([128, EC, NP - N]),
                )
                for t in range(NT):
                    nr = ROWS[t]
                    xnat = xnat_pool.tile([nr, E], F32, tag="xnat")
                    nc.sync.dma_start(
                        out=xnat[:, :], in_=x_d[b, t * 128 : t * 128 + nr, :]
                    )
                    for g in range(2):
                        xtp = pss_pool.tile([128, 3, nr], F32, tag="pss")
                        for j in range(3):
                            c = 3 * g + j
                            nc.tensor.matmul(
                                xtp[:, j, :],
                                xnat[:, c * 128 : (c + 1) * 128],
                                ident[:nr, :nr],
                                is_transpose=True,
                            )
                        nc.vector.tensor_copy(
                            xt6[:, 3 * g : 3 * g + 3, t * 128 : t * 128 + nr],
                            xtp[:, :, :],
                        )
                        yield None

                qt = qt_pool.tile([D, H * NP], F16, tag="qt")
                kt = qt_pool.tile([D, H * NP], F16, tag="kt")
                for dst, w_sb, b_sb in ((qt, wq_sb, bq_sb), (kt, wk_sb, bk_sb)):
                    for h in range(H):
                        for q0, q1 in PSPLITS:
                            p_ps = pss_pool.tile([D, q1 - q0], F32, tag="pss")
                            for c in range(EC):
                                nc.tensor.matmul(
                                    p_ps[:, :],
                                    w_sb[:, c * E + h * D : c * E + (h + 1) * D],
                                    xt6[:, c, q0:q1],
                                    start=(c == 0),
                                    stop=(c == EC - 1),
                                )
                            nc.vector.tensor_scalar_add(
                                dst[:, h * NP + q0 : h * NP + q1],
                                p_ps[:, :],
                                b_sb[:, h : h + 1],
                            )
                            yield None

                # V (pre-scaled by host): vaug [128, NT, H, D+1], col D = ones
                vaug = v_pool.tile([128, NT, H, DA], F32R, tag="vaug")
                nc.vector.tensor_copy(
                    vaug[:, :, :, D : D + 1],
                    one1[:, 0:1]
                    .unsqueeze(1)
                    .unsqueeze(1)
                    .broadcast_to([128, NT, H, 1]),
                )
                for t in range(NT):
                    nr = ROWS[t]
                    for vi, (e0, e1) in enumerate(VSPLITS):
                        v_ps = pss_pool.tile([nr, e1 - e0], F32, tag="pss")
                        for c in range(EC):
                            nc.tensor.matmul(
                                v_ps[:, :],
                                xt6[:, c, t * 128 : t * 128 + nr],
                                wv_sb[:, c * E + e0 : c * E + e1],
                                start=(c == 0),
                                stop=(c == EC - 1),
                            )
                        h0 = vi * (H // 2)
                        nc.vector.tensor_add(
                            vaug[:nr, t, h0 : h0 + H // 2, 0:D],
                            v_ps.rearrange("p (h d) -> p h d", h=H // 2),
                            bv_sb[:nr, e0:e1].rearrange("p (h d) -> p h d", h=H // 2),
                        )
                        yield None
                yield (qt, kt, vaug)

            def p2(b, tiles, filler, result_box):
                """Attention for batch b; pumps `filler` between energy tiles."""
                qt, kt, vaug = tiles
                out_sb = out_pool.tile([128, NT, E], F32, tag="out")
                ee = {}  # (h, j) -> expE tile
                at = {}  # h -> attn^T sbuf tile
                av = {}  # h -> AV psum tile

                def pump(n=1):
                    for _ in range(n):
                        try:
                            r = next(filler)
                            if r is not None:
                                result_box["tiles"] = r
                        except StopIteration:
                            return

                def emit_e(h, j):
                    nr = ROWS[j]
                    e_ps = psb_pool.tile([nr, NP], F32, tag="psb")
                    lhsT = kt[:, h * NP + j * 128 : h * NP + j * 128 + nr]
                    for q0, q1 in QSPLITS:
                        nc.tensor.matmul(
                            e_ps[:, q0:q1], lhsT, qt[:, h * NP + q0 : h * NP + q1]
                        )
                    t = ee_pool.tile([nr, NP], F32R, tag="ee", name="ee_t")
                    nc.scalar.activation(t[:, :], e_ps[:, :], Exp)
                    ee[(h, j)] = t

                def emit_av(h, j):
                    if j == 0:
                        av[h] = psb_pool.tile([DA, NP], F32, tag="psb", name="av_ps")
                    nr = ROWS[j]
                    for q0, q1 in QSPLITS:
                        nc.tensor.matmul(
                            av[h][:, q0:q1],
                            vaug[:nr, j, h, :],
                            ee[(h, j)][:, q0:q1],
                            start=(j == 0),
                            stop=(j == NT - 1),
                        )
                    if j == NT - 1:
                        for jj in range(NT):
                            del ee[(h, jj)]

                def emit_at_copy(h):
                    at[h] = at_pool.tile([DA, NP], F32, tag="at", name="at_sb")
                    nc.vector.tensor_copy(at[h][:, :], av.pop(h)[:, :])

                def emit_tr_norm(h):
                    at_sb = at.pop(h)
                    tr_ps = pss_pool.tile([128, NT, DA], F32, tag="pss")
                    for t in range(NT):
                        nr = ROWS[t]
                        nc.tensor.matmul(
                            tr_ps[:nr, t, :],
                            at_sb[:, t * 128 : t * 128 + nr],
                            ident[:DA, :DA],
                            is_transpose=True,
                        )
                    r = r_pool.tile([128, NT], F32, tag="r")
                    nc.vector.reciprocal(r[:, 0 : NT - 1], tr_ps[:, 0 : NT - 1, D])
                    nc.vector.reciprocal(
                        r[: ROWS[-1], NT - 1 : NT], tr_ps[: ROWS[-1], NT - 1, D : D + 1]
                    )
                    nc.vector.tensor_mul(
                        out_sb[:, 0 : NT - 1, h * D : (h + 1) * D],
                        tr_ps[:, 0 : NT - 1, 0:D],
                        r[:, 0 : NT - 1].unsqueeze(2).broadcast_to([128, NT - 1, D]),
                    )
                    nr = ROWS[-1]
                    nc.vector.tensor_mul(
                        out_sb[:nr, NT - 1, h * D : (h + 1) * D],
                        tr_ps[:nr, NT - 1, 0:D],
                        r[:nr, NT - 1 : NT].broadcast_to([nr, D]),
                    )

                for h in range(H):
                    for j in range(NT):
                        emit_e(h, j)
                        if h >= 2:
                            emit_av(h - 2, j)
                        pump(1)
                    if h >= 2:
                        emit_at_copy(h - 2)
                    if h >= 3:
                        emit_tr_norm(h - 3)
                # tail: AV for heads H-2, H-1; transposes for H-3..H-1
                for h in (H - 2, H - 1):
                    for j in range(NT):
                        emit_av(h, j)
                        pump(1)
                    emit_at_copy(h)
                    emit_tr_norm(h - 1)
                emit_tr_norm(H - 1)
                pump(1000)  # drain any remaining filler

                for t in range(NT):
                    nr = ROWS[t]
                    nc.sync.dma_start(
                        out=o_d[b, t * 128 : t * 128 + nr, :], in_=out_sb[:nr, t, :]
                    )

            box0 = {}
            for r in p01_units(0):
                if r is not None:
                    box0["tiles"] = r
            tiles = box0["tiles"]
            for b in range(nb):
                box = {}
                filler = p01_units(b + 1) if b + 1 < nb else iter(())
                p2(b, tiles, filler, box)
                if b + 1 < nb:
                    tiles = box.get("tiles")
                    assert tiles is not None, "filler not fully pumped"
    nc.compile()
    return nc


_CACHE = {}


def _get_nc(nb=NB):
    if nb not in _CACHE:
        _CACHE[nb] = build(nb)
    return _CACHE[nb]


def kernel(x, Wq, bq, Wk, bk, Wv, bv):
    s = float(D) ** -0.5
    x = np.ascontiguousarray(np.asarray(x, dtype=np.float32))
    in_common = {
        "wq": np.ascontiguousarray(np.asarray(Wq, dtype=np.float16)),
        "wk": np.ascontiguousarray(np.asarray(Wk, dtype=np.float16)),
        "wv": np.ascontiguousarray(
            (np.asarray(Wv, dtype=np.float32) * s).astype(np.float16)
        ),
        "bq": np.ascontiguousarray(np.asarray(bq, dtype=np.float32)),
        "bk": np.ascontiguousarray(np.asarray(bk, dtype=np.float32)),
        "bv": np.ascontiguousarray(np.asarray(bv, dtype=np.float32) * s),
    }
    nc = _get_nc()
    in_maps = [
        {"x": x[c * NB : (c + 1) * NB], **in_common} for c in range(NCORES)
    ]
    res = run_bass_kernel_spmd(nc, in_maps, list(range(NCORES))).results
    return np.concatenate([r["out"] for r in res], axis=0)


# revision 9
# speedup vs baseline: 1.7687x; 1.0978x over previous
"""Multi-head attention (B=64, N=577, E=768, H=8) on 8 Trainium2 NeuronCores.

Sharding: data-parallel over batch — each core gets 8 batches, full weights.

Per-core dataflow (per batch):
  x [577,768] --load--> SBUF, PE-transpose -> xT [768(6x128), 577]
  QT = Wq.T @ xT  (head-packed [96, 8*578]), KT likewise        (PE, fp16)
  V  = xT.T @ Wv + bv, scaled by 1/sqrt(D), ones-column per head (-> rowsums)
  per head h:
    eT[k,q] = KT_h.T @ QT_h   (k on partitions, 5 k-tiles)      (PE, fp16)
    expE = exp(eT)            (no max-subtraction; |e| <~ 19)   (ACT, ->f32r)
    aT[d,q] (+rowsum row) = sum_k V_h[k,d+1] * expE[k,q]        (PE, f32r)
    attn[q,d] = transpose(aT) / rowsum                          (PE + DVE)
  out[b] assembled in SBUF [128,5,768], DMA'd back.

softmax(e)*scaling @ (x Wv + bv) == (exp(e) @ (s*(x Wv + bv))) / rowsum(exp(e)),
so the host passes Wv*s, bv*s (as fp16) and the kernel never multiplies by s.

Scheduling: the attention phase is rate-limited by ACT's exp (~740ns per
[128,578] tile) — PE produces an energy tile in ~540ns, so a naive ordering
stalls PE on PSUM slots every k-tile and the HAM clock gate drops the PE to
1.2 GHz. To keep PE streak-busy, batch b+1's x-load/transpose and QKV
projection matmuls are interleaved as filler work between batch b's
energy/AV matmuls (generator `p01_units` pumped inside the head loop).

fp32r matmul ISA restrictions: moving-operand and psum-dst innermost counts
must be even, dst 8B-aligned, start_partition 0 — hence the N->578 padded
free dim (pad column zeroed) and the even projection splits.
"""

import numpy as np
from contextlib import ExitStack

import concourse.bass as bass
import concourse.bacc as bacc
import concourse.tile as tile
from concourse import mybir, masks
from concourse.bass_utils import run_bass_kernel_spmd

F32 = mybir.dt.float32
F32R = mybir.dt.float32r
F16 = mybir.dt.float16
Exp = mybir.ActivationFunctionType.Exp

B, N, E, H = 64, 577, 768, 8
D = E // H  # 96
NCORES = 8
NB = B // NCORES  # batches per core
NP = 578  # N padded to even (fp32r matmul moving/dst counts must be even)
NT = (N + 127) // 128  # 5 row tiles
ROWS = [min(128, N - t * 128) for t in range(NT)]  # [128,128,128,128,65]
EC = E // 128  # 6 contraction chunks
QSPLITS = [(0, 512), (512, NP)]  # 2-bank psum column splits (energy / AV)
PSPLITS = [(0, 290), (290, NP)]  # 1-bank psum splits for Q/K projections
VSPLITS = [(0, 384), (384, E)]  # 1-bank psum splits for V projection
DA = D + 1  # V columns per head incl. ones column


def build(nb=NB):
    nc = bacc.Bacc("TRN2", num_devices=NCORES)
    x_d = nc.declare_dram_parameter("x", [nb, E, N], F16, isOutput=False).ap()
    wq_d = nc.declare_dram_parameter("wq", [E, E], F16, isOutput=False).ap()
    wk_d = nc.declare_dram_parameter("wk", [E, E], F16, isOutput=False).ap()
    wv_d = nc.declare_dram_parameter("wv", [E, E], F16, isOutput=False).ap()
    bq_d = nc.declare_dram_parameter("bq", [E], F32, isOutput=False).ap()
    bk_d = nc.declare_dram_parameter("bk", [E], F32, isOutput=False).ap()
    bv_d = nc.declare_dram_parameter("bv", [E], F32, isOutput=False).ap()
    o_d = nc.declare_dram_parameter("out", [nb, N, E], F32, isOutput=True).ap()

    with tile.TileContext(nc) as tc:
        with ExitStack() as ctx:
            cpool = ctx.enter_context(tc.tile_pool(name="cpool", bufs=1))
            xt_pool = ctx.enter_context(tc.tile_pool(name="xt_pool", bufs=2))
            qt_pool = ctx.enter_context(tc.tile_pool(name="qt_pool", bufs=2))
            v_pool = ctx.enter_context(tc.tile_pool(name="v_pool", bufs=2))
            ee_pool = ctx.enter_context(tc.tile_pool(name="ee_pool", bufs=15))
            at_pool = ctx.enter_context(tc.tile_pool(name="at_pool", bufs=3))
            r_pool = ctx.enter_context(tc.tile_pool(name="r_pool", bufs=2))
            out_pool = ctx.enter_context(tc.tile_pool(name="out_pool", bufs=2))
            # "big" = 2-bank tiles (energy, AV); "small" = 1-bank tiles
            # (x-transpose blocks, attn-transpose blocks, projection halves)
            psb_pool = ctx.enter_context(
                tc.tile_pool(name="psb_pool", bufs=3, space="PSUM")
            )
            pss_pool = ctx.enter_context(
                tc.tile_pool(name="pss_pool", bufs=2, space="PSUM")
            )

            # ---- constants ----
            ident = cpool.tile([128, 128], F32)
            masks.make_identity(nc, ident[:, :])
            zero1 = cpool.tile([128, 1], F32)
            nc.gpsimd.memset(zero1[:, :], 0.0)
            one1 = cpool.tile([128, 1], F32)
            nc.gpsimd.memset(one1[:, :], 1.0)

            wq_sb = cpool.tile([128, EC * E], F16)
            wk_sb = cpool.tile([128, EC * E], F16)
            wv_sb = cpool.tile([128, EC * E], F16)
            for w_sb, w_d in ((wq_sb, wq_d), (wk_sb, wk_d), (wv_sb, wv_d)):
                for c in range(EC):
                    nc.sync.dma_start(
                        out=w_sb[:, c * E : (c + 1) * E],
                        in_=w_d[c * 128 : (c + 1) * 128, :],
                    )
            bq_sb = cpool.tile([D, H], F32)
            bk_sb = cpool.tile([D, H], F32)
            nc.sync.dma_start(out=bq_sb[:, :], in_=bq_d.rearrange("(h d) -> d h", h=H))
            nc.sync.dma_start(out=bk_sb[:, :], in_=bk_d.rearrange("(h d) -> d h", h=H))
            bv_sb = cpool.tile([128, E], F32)
            nc.sync.dma_start(out=bv_sb[:, :], in_=bv_d.partition_broadcast(128))

            def p01_units(b):
                """Load/transpose x and project Q/K/V for batch b.

                Generator yielding after each small chunk of PE work
                (~3-6 matmuls) so it can be pumped as filler between the
                previous batch's attention matmuls. The final yield carries
                the produced (qt, kt, vaug) tiles.
                """
                xt = xt_pool.tile([128, EC * NP], F16, tag="xt")
                xt6 = xt.rearrange("p (c n) -> p c n", c=EC)
                nc.vector.tensor_copy(
                    xt6[:, :, N:NP],
                    zero1[:, 0:1].unsqueeze(1).broadcast_to([128, EC, NP - N]),
                )
                for c in range(EC):
                    nc.sync.dma_start(
                        out=xt6[:, c, 0:N], in_=x_d[b, c * 128 : (c + 1) * 128, :]
                    )

                qt = qt_pool.tile([D, H * NP], F16, tag="qt")
                kt = qt_pool.tile([D, H * NP], F16, tag="kt")
                for dst, w_sb, b_sb in ((qt, wq_sb, bq_sb), (kt, wk_sb, bk_sb)):
                    for h in range(H):
                        for q0, q1 in PSPLITS:
                            p_ps = pss_pool.tile([D, q1 - q0], F32, tag="pss")
                            for c in range(EC):
                                nc.tensor.matmul(
                                    p_ps[:, :],
                                    w_sb[:, c * E + h * D : c * E + (h + 1) * D],
                                    xt6[:, c, q0:q1],
                                    start=(c == 0),
                                    stop=(c == EC - 1),
                                )
                            nc.vector.tensor_scalar_add(
                                dst[:, h * NP + q0 : h * NP + q1],
                                p_ps[:, :],
                                b_sb[:, h : h + 1],
                            )
                            yield None

                # V (pre-scaled by host): vaug [128, NT, H, D+1], col D = ones
                vaug = v_pool.tile([128, NT, H, DA], F32R, tag="vaug")
                nc.vector.tensor_copy(
                    vaug[:, :, :, D : D + 1],
                    one1[:, 0:1]
                    .unsqueeze(1)
                    .unsqueeze(1)
                    .broadcast_to([128, NT, H, 1]),
                )
                for t in range(NT):
                    nr = ROWS[t]
                    for vi, (e0, e1) in enumerate(VSPLITS):
                        v_ps = pss_pool.tile([nr, e1 - e0], F32, tag="pss")
                        for c in range(EC):
                            nc.tensor.matmul(
                                v_ps[:, :],
                                xt6[:, c, t * 128 : t * 128 + nr],
                                wv_sb[:, c * E + e0 : c * E + e1],
                                start=(c == 0),
                                stop=(c == EC - 1),
                            )
                        h0 = vi * (H // 2)
                        nc.vector.tensor_add(
                            vaug[:nr, t, h0 : h0 + H // 2, 0:D],
                            v_ps.rearrange("p (h d) -> p h d", h=H // 2),
                            bv_sb[:nr, e0:e1].rearrange("p (h d) -> p h d", h=H // 2),
                        )
                        yield None
                yield (qt, kt, vaug)

            def p2(b, tiles, filler, result_box):
                """Attention for batch b; pumps `filler` between energy tiles."""
                qt, kt, vaug = tiles
                out_sb = out_pool.tile([128, NT, E], F32, tag="out")
                ee = {}  # (h, j) -> expE tile
                at = {}  # h -> attn^T sbuf tile
                av = {}  # h -> AV psum tile

                def pump(n=1):
                    for _ in range(n):
                        try:
                            r = next(filler)
                            if r is not None:
                                result_box["tiles"] = r
                        except StopIteration:
                            return

                def emit_e(h, j):
                    nr = ROWS[j]
                    e_ps = psb_pool.tile([nr, NP], F32, tag="psb")
                    lhsT = kt[:, h * NP + j * 128 : h * NP + j * 128 + nr]
                    for q0, q1 in QSPLITS:
                        nc.tensor.matmul(
                            e_ps[:, q0:q1], lhsT, qt[:, h * NP + q0 : h * NP + q1]
                        )
                    t = ee_pool.tile([nr, NP], F32R, tag="ee", name="ee_t")
                    nc.scalar.activation(t[:, :], e_ps[:, :], Exp)
                    ee[(h, j)] = t

                def emit_av(h, j):
                    if j == 0:
                        av[h] = psb_pool.tile([DA, NP], F32, tag="psb", name="av_ps")
                    nr = ROWS[j]
                    for q0, q1 in QSPLITS:
                        nc.tensor.matmul(
                            av[h][:, q0:q1],
                            vaug[:nr, j, h, :],
                            ee[(h, j)][:, q0:q1],
                            start=(j == 0),
                            stop=(j == NT - 1),
                        )
                    if j == NT - 1:
                        for jj in range(NT):
                            del ee[(h, jj)]

                def emit_at_copy(h):
                    at[h] = at_pool.tile([DA, NP], F32, tag="at", name="at_sb")
                    nc.vector.tensor_copy(at[h][:, :], av.pop(h)[:, :])

                def emit_tr_norm(h):
                    at_sb = at.pop(h)
                    tr_ps = pss_pool.tile([128, NT, DA], F32, tag="pss")
                    for t in range(NT):
                        nr = ROWS[t]
                        nc.tensor.matmul(
                            tr_ps[:nr, t, :],
                            at_sb[:, t * 128 : t * 128 + nr],
                            ident[:DA, :DA],
                            is_transpose=True,
                        )
                    r = r_pool.tile([128, NT], F32, tag="r")
                    nc.vector.reciprocal(r[:, 0 : NT - 1], tr_ps[:, 0 : NT - 1, D])
                    nc.vector.reciprocal(
                        r[: ROWS[-1], NT - 1 : NT], tr_ps[: ROWS[-1], NT - 1, D : D + 1]
                    )
                    nc.vector.tensor_mul(
                        out_sb[:, 0 : NT - 1, h * D : (h + 1) * D],
                        tr_ps[:, 0 : NT - 1, 0:D],
                        r[:, 0 : NT - 1].unsqueeze(2).broadcast_to([128, NT - 1, D]),
                    )
                    nr = ROWS[-1]
                    nc.vector.tensor_mul(
                        out_sb[:nr, NT - 1, h * D : (h + 1) * D],
                        tr_ps[:nr, NT - 1, 0:D],
                        r[:nr, NT - 1 : NT].broadcast_to([nr, D]),
                    )

                for h in range(H):
                    for j in range(NT):
                        emit_e(h, j)
                        if h >= 2:
                            emit_av(h - 2, j)
                        pump(1)
                    if h >= 2:
                        emit_at_copy(h - 2)
                    if h >= 3:
                        emit_tr_norm(h - 3)
                # tail: AV for heads H-2, H-1; transposes for H-3..H-1
                for h in (H - 2, H - 1):
                    for j in range(NT):
                        emit_av(h, j)
                        pump(1)
                    emit_at_copy(h)
                    emit_tr_norm(h - 1)
                emit_tr_norm(H - 1)
                pump(1000)  # drain any remaining filler

                for t in range(NT):
                    nr = ROWS[t]
                    nc.sync.dma_start(
                        out=o_d[b, t * 128 : t * 128 + nr, :], in_=out_sb[:nr, t, :]
                    )

            box0 = {}
            for r in p01_units(0):
                if r is not None:
                    box0["tiles"] = r
            tiles = box0["tiles"]
            for b in range(nb):
                box = {}
                filler = p01_units(b + 1) if b + 1 < nb else iter(())
                p2(b, tiles, filler, box)
                if b + 1 < nb:
                    tiles = box.get("tiles")
                    assert tiles is not None, "filler not fully pumped"
    nc.compile()
    return nc


_CACHE = {}


def _get_nc(nb=NB):
    if nb not in _CACHE:
        _CACHE[nb] = build(nb)
    return _CACHE[nb]


def prepare_in_maps(x, Wq, bq, Wk, bk, Wv, bv):
    s = float(D) ** -0.5
    # x enters the kernel pre-transposed ([b, E, N]) in fp16 — the kernel
    # rounds xT to fp16 anyway, so this is numerically identical and saves
    # all on-chip transposes.
    xt = np.ascontiguousarray(
        np.asarray(x, dtype=np.float32).transpose(0, 2, 1).astype(np.float16)
    )
    in_common = {
        "wq": np.ascontiguousarray(np.asarray(Wq, dtype=np.float16)),
        "wk": np.ascontiguousarray(np.asarray(Wk, dtype=np.float16)),
        "wv": np.ascontiguousarray(
            (np.asarray(Wv, dtype=np.float32) * s).astype(np.float16)
        ),
        "bq": np.ascontiguousarray(np.asarray(bq, dtype=np.float32)),
        "bk": np.ascontiguousarray(np.asarray(bk, dtype=np.float32)),
        "bv": np.ascontiguousarray(np.asarray(bv, dtype=np.float32) * s),
    }
    return [
        {"x": np.ascontiguousarray(xt[c * NB : (c + 1) * NB]), **in_common}
        for c in range(NCORES)
    ]


def kernel(x, Wq, bq, Wk, bk, Wv, bv):
    nc = _get_nc()
    in_maps = prepare_in_maps(x, Wq, bq, Wk, bk, Wv, bv)
    res = run_bass_kernel_spmd(nc, in_maps, list(range(NCORES))).results
    return np.concatenate([r["out"] for r in res], axis=0)


# revision 14
# speedup vs baseline: 2.0377x; 1.1521x over previous
"""Multi-head attention (B=64, N=577, E=768, H=8) on 8 Trainium2 NeuronCores.

Sharding: data-parallel over batch — each core gets 8 batches, full weights.

Per-core dataflow (per batch):
  x [577,768] --load--> SBUF, PE-transpose -> xT [768(6x128), 577]
  QT = Wq.T @ xT  (head-packed [96, 8*578]), KT likewise        (PE, fp16)
  V  = xT.T @ Wv + bv, scaled by 1/sqrt(D), ones-column per head (-> rowsums)
  per head h:
    eT[k,q] = KT_h.T @ QT_h   (k on partitions, 5 k-tiles)      (PE, fp16)
    expE = exp(eT)            (no max-subtraction; |e| <~ 19)   (ACT, ->f32r)
    aT[d,q] (+rowsum row) = sum_k V_h[k,d+1] * expE[k,q]        (PE, f32r)
    attn[q,d] = transpose(aT) / rowsum                          (PE + DVE)
  out[b] assembled in SBUF [128,5,768], DMA'd back.

softmax(e)*scaling @ (x Wv + bv) == (exp(e) @ (s*(x Wv + bv))) / rowsum(exp(e)),
so the host passes Wv*s, bv*s (as fp16) and the kernel never multiplies by s.

Scheduling: the attention phase is rate-limited by ACT's exp (~740ns per
[128,578] tile) — PE produces an energy tile in ~540ns, so a naive ordering
stalls PE on PSUM slots every k-tile and the HAM clock gate drops the PE to
1.2 GHz. To keep PE streak-busy, batch b+1's x-load/transpose and QKV
projection matmuls are interleaved as filler work between batch b's
energy/AV matmuls (generator `p01_units` pumped inside the head loop).

fp32r matmul ISA restrictions: moving-operand and psum-dst innermost counts
must be even, dst 8B-aligned, start_partition 0 — hence the N->578 padded
free dim (pad column zeroed) and the even projection splits.
"""

import os
import numpy as np
from contextlib import ExitStack

import concourse.bass as bass
import concourse.bacc as bacc
import concourse.tile as tile
from concourse import mybir, masks
from concourse.bass_utils import run_bass_kernel_spmd


F32 = mybir.dt.float32
F32R = mybir.dt.float32r
F16 = mybir.dt.float16
BF16 = mybir.dt.bfloat16
Exp = mybir.ActivationFunctionType.Exp

B, N, E, H = 64, 577, 768, 8
D = E // H  # 96
NCORES = 8
NB = B // NCORES  # batches per core
NP = 578  # N padded to even (fp32r matmul moving/dst counts must be even)
NT = (N + 127) // 128  # 5 row tiles
ROWS = [min(128, N - t * 128) for t in range(NT)]  # [128,128,128,128,65]
EC = E // 128  # 6 contraction chunks
QSPLITS = [(0, 512), (512, NP)]  # 2-bank psum column splits (energy / AV)
PSPLITS = [(0, 290), (290, NP)]  # 1-bank psum splits for Q/K projections
VSPLITS = [(0, 384), (384, E)]  # 1-bank psum splits for V projection
DA = D + 1  # V columns per head incl. ones column
QCH = [(t * 128, min(128, NP - t * 128)) for t in range(NT)]  # q-chunks of 128


def build(nb=NB):
    nc = bacc.Bacc("TRN2", num_devices=NCORES)
    x_d = nc.declare_dram_parameter("x", [nb, E, N], F16, isOutput=False).ap()
    wq_d = nc.declare_dram_parameter("wq", [E, E], F16, isOutput=False).ap()
    wk_d = nc.declare_dram_parameter("wk", [E, E], F16, isOutput=False).ap()
    wv_d = nc.declare_dram_parameter("wv", [E, E], F16, isOutput=False).ap()
    bq_d = nc.declare_dram_parameter("bq", [E], F32, isOutput=False).ap()
    bk_d = nc.declare_dram_parameter("bk", [E], F32, isOutput=False).ap()
    bv_d = nc.declare_dram_parameter("bv", [E], F32, isOutput=False).ap()
    o_d = nc.declare_dram_parameter("out", [nb, N, E], F32, isOutput=True).ap()

    with tile.TileContext(nc) as tc:
        with ExitStack() as ctx:
            cpool = ctx.enter_context(tc.tile_pool(name="cpool", bufs=1))
            xt_pool = ctx.enter_context(tc.tile_pool(name="xt_pool", bufs=2))
            qt_pool = ctx.enter_context(tc.tile_pool(name="qt_pool", bufs=2))
            v_pool = ctx.enter_context(tc.tile_pool(name="v_pool", bufs=2))
            ee_pool = ctx.enter_context(tc.tile_pool(name="ee_pool", bufs=15))
            r_pool = ctx.enter_context(tc.tile_pool(name="r_pool", bufs=2))
            out_pool = ctx.enter_context(tc.tile_pool(name="out_pool", bufs=2))
            # "big" = 2-bank tiles (energy, AV); "small" = 1-bank tiles
            # (x-transpose blocks, attn-transpose blocks, projection halves)
            psb_pool = ctx.enter_context(
                tc.tile_pool(name="psb_pool", bufs=2, space="PSUM")
            )
            pss_pool = ctx.enter_context(
                tc.tile_pool(name="pss_pool", bufs=4, space="PSUM")
            )

            # ---- constants ----
            ident = cpool.tile([128, 128], F32)
            masks.make_identity(nc, ident[:, :])
            zero1 = cpool.tile([128, 1], F32)
            nc.gpsimd.memset(zero1[:, :], 0.0)
            one1 = cpool.tile([128, 1], F32)
            nc.gpsimd.memset(one1[:, :], 1.0)

            wq_sb = cpool.tile([128, EC * E], F16)
            wk_sb = cpool.tile([128, EC * E], F16)
            wv_sb = cpool.tile([128, EC * E], F16)
            bq_sb = cpool.tile([D, H], F32)
            bk_sb = cpool.tile([D, H], F32)
            bv_sb = cpool.tile([128, E], F32)

            def dma_weights():
                # Q/K weights first (first projection needs them); V last.
                for c in range(EC):
                    nc.sync.dma_start(
                        out=wq_sb[:, c * E : (c + 1) * E],
                        in_=wq_d[c * 128 : (c + 1) * 128, :],
                    )
                nc.sync.dma_start(
                    out=bq_sb[:, :], in_=bq_d.rearrange("(h d) -> d h", h=H)
                )
                for c in range(EC):
                    nc.sync.dma_start(
                        out=wk_sb[:, c * E : (c + 1) * E],
                        in_=wk_d[c * 128 : (c + 1) * 128, :],
                    )
                nc.sync.dma_start(
                    out=bk_sb[:, :], in_=bk_d.rearrange("(h d) -> d h", h=H)
                )
                nc.sync.dma_start(out=bv_sb[:, :], in_=bv_d.partition_broadcast(128))
                for c in range(EC):
                    nc.sync.dma_start(
                        out=wv_sb[:, c * E : (c + 1) * E],
                        in_=wv_d[c * 128 : (c + 1) * 128, :],
                    )

            def p01_units(b, first=False):
                """Load/transpose x and project Q/K/V for batch b.

                Generator yielding after each small chunk of PE work
                (~3-6 matmuls) so it can be pumped as filler between the
                previous batch's attention matmuls. The final yield carries
                the produced (qt, kt, vaug) tiles.
                """
                xt = xt_pool.tile([128, EC * NP], F16, tag="xt")
                xt6 = xt.rearrange("p (c n) -> p c n", c=EC)
                nc.vector.tensor_copy(
                    xt6[:, :, N:NP],
                    zero1[:, 0:1].unsqueeze(1).broadcast_to([128, EC, NP - N]),
                )
                for c in range(EC):
                    nc.sync.dma_start(
                        out=xt6[:, c, 0:N], in_=x_d[b, c * 128 : (c + 1) * 128, :]
                    )
                if first:
                    dma_weights()

                qt = qt_pool.tile([D, H * NP], F16, tag="qt")
                kt = qt_pool.tile([D, H * NP], F16, tag="kt")
                for dst, w_sb, b_sb in ((qt, wq_sb, bq_sb), (kt, wk_sb, bk_sb)):
                    for h in range(H):
                        for q0, q1 in PSPLITS:
                            p_ps = pss_pool.tile([D, q1 - q0], F32, tag="pss")
                            for c in range(EC):
                                nc.tensor.matmul(
                                    p_ps[:, :],
                                    w_sb[:, c * E + h * D : c * E + (h + 1) * D],
                                    xt6[:, c, q0:q1],
                                    start=(c == 0),
                                    stop=(c == EC - 1),
                                )
                            nc.vector.tensor_scalar_add(
                                dst[:, h * NP + q0 : h * NP + q1],
                                p_ps[:, :],
                                b_sb[:, h : h + 1],
                            )
                            yield None

                # V (pre-scaled by host): vaug [128, NT, H, D+1], col D = ones
                vaug = v_pool.tile([128, NT, H, DA], BF16, tag="vaug")
                nc.vector.tensor_copy(
                    vaug[:, :, :, D : D + 1],
                    one1[:, 0:1]
                    .unsqueeze(1)
                    .unsqueeze(1)
                    .broadcast_to([128, NT, H, 1]),
                )
                for t in range(NT):
                    nr = ROWS[t]
                    for vi, (e0, e1) in enumerate(VSPLITS):
                        v_ps = pss_pool.tile([nr, e1 - e0], F32, tag="pss")
                        for c in range(EC):
                            nc.tensor.matmul(
                                v_ps[:, :],
                                xt6[:, c, t * 128 : t * 128 + nr],
                                wv_sb[:, c * E + e0 : c * E + e1],
                                start=(c == 0),
                                stop=(c == EC - 1),
                            )
                        h0 = vi * (H // 2)
                        nc.vector.tensor_add(
                            vaug[:nr, t, h0 : h0 + H // 2, 0:D],
                            v_ps.rearrange("p (h d) -> p h d", h=H // 2),
                            bv_sb[:nr, e0:e1].rearrange("p (h d) -> p h d", h=H // 2),
                        )
                        yield None
                yield (qt, kt, vaug)

            def p2(b, tiles, filler, result_box):
                """Attention for batch b; pumps `filler` between energy tiles."""
                qt, kt, vaug = tiles
                out_sb = out_pool.tile([128, NT, E], F32, tag="out")
                ee = {}  # (h, j) -> expE tile
                av = {}  # h -> AV psum tile [128, NT, DA] (q on partitions)

                def pump(n=1):
                    for _ in range(n):
                        try:
                            r = next(filler)
                            if r is not None:
                                result_box["tiles"] = r
                        except StopIteration:
                            return

                def emit_e(h, j):
                    nr = ROWS[j]
                    e_ps = psb_pool.tile([nr, NP], F32, tag="psb")
                    lhsT = kt[:, h * NP + j * 128 : h * NP + j * 128 + nr]
                    for q0, q1 in QSPLITS:
                        nc.tensor.matmul(
                            e_ps[:, q0:q1], lhsT, qt[:, h * NP + q0 : h * NP + q1]
                        )
                    t = ee_pool.tile([nr, NP], BF16, tag="ee", name="ee_t")
                    nc.scalar.activation(t[:, :], e_ps[:, :], Exp)
                    ee[(h, j)] = t

                def emit_av(h, t):
                    # attn[q, d] directly for q-chunk t: lhsT = expE chunk
                    # (stationary), rhs = V_h column block (97 cols incl
                    # ones). One complete accumulation chain per call — PSUM
                    # groups are bank-granular, so chains in the shared bank
                    # must not interleave.
                    if t == 0:
                        av[h] = pss_pool.tile(
                            [128, NT, DA], F32, tag="pss", name="av_ps"
                        )
                    c0, cw = QCH[t]
                    for j in range(NT):
                        nr = ROWS[j]
                        nc.tensor.matmul(
                            av[h][:cw, t, :],
                            ee[(h, j)][:, c0 : c0 + cw],
                            vaug[:nr, j, h, :],
                            start=(j == 0),
                            stop=(j == NT - 1),
                        )
                    if t == NT - 1:
                        for jj in range(NT):
                            del ee[(h, jj)]

                def emit_norm(h):
                    p = av.pop(h)
                    r = r_pool.tile([128, NT], F32, tag="r")
                    nc.vector.reciprocal(r[:, 0 : NT - 1], p[:, 0 : NT - 1, D])
                    nc.vector.reciprocal(
                        r[: ROWS[-1], NT - 1 : NT], p[: ROWS[-1], NT - 1, D : D + 1]
                    )
                    nc.vector.tensor_mul(
                        out_sb[:, 0 : NT - 1, h * D : (h + 1) * D],
                        p[:, 0 : NT - 1, 0:D],
                        r[:, 0 : NT - 1].unsqueeze(2).broadcast_to([128, NT - 1, D]),
                    )
                    nr = ROWS[-1]
                    nc.vector.tensor_mul(
                        out_sb[:nr, NT - 1, h * D : (h + 1) * D],
                        p[:nr, NT - 1, 0:D],
                        r[:nr, NT - 1 : NT].broadcast_to([nr, D]),
                    )

                for h in range(H):
                    for j in range(NT):
                        emit_e(h, j)
                        if h >= 2:
                            emit_av(h - 2, j)
                        pump(1)
                    if h >= 2:
                        emit_norm(h - 2)
                # tail: AV + norm for heads H-2, H-1
                for h in (H - 2, H - 1):
                    for j in range(NT):
                        emit_av(h, j)
                        pump(1)
                    emit_norm(h)
                pump(1000)  # drain any remaining filler

                for t in range(NT):
                    nr = ROWS[t]
                    nc.sync.dma_start(
                        out=o_d[b, t * 128 : t * 128 + nr, :], in_=out_sb[:nr, t, :]
                    )

            box0 = {}
            for r in p01_units(0, first=True):
                if r is not None:
                    box0["tiles"] = r
            tiles = box0["tiles"]
            for b in range(nb):
                box = {}
                filler = p01_units(b + 1) if b + 1 < nb else iter(())
                p2(b, tiles, filler, box)
                if b + 1 < nb:
                    tiles = box.get("tiles")
                    assert tiles is not None, "filler not fully pumped"
    nc.compile()
    return nc


_CACHE = {}


def _get_nc(nb=NB):
    if nb not in _CACHE:
        _CACHE[nb] = build(nb)
    return _CACHE[nb]


def prepare_in_maps(x, Wq, bq, Wk, bk, Wv, bv):
    s = float(D) ** -0.5
    # x enters the kernel pre-transposed ([b, E, N]) in fp16 — the kernel
    # rounds xT to fp16 anyway, so this is numerically identical and saves
    # all on-chip transposes.
    xt = np.ascontiguousarray(
        np.asarray(x, dtype=np.float32).transpose(0, 2, 1).astype(np.float16)
    )
    in_common = {
        "wq": np.ascontiguousarray(np.asarray(Wq, dtype=np.float16)),
        "wk": np.ascontiguousarray(np.asarray(Wk, dtype=np.float16)),
        "wv": np.ascontiguousarray(
            (np.asarray(Wv, dtype=np.float32) * s).astype(np.float16)
        ),
        "bq": np.ascontiguousarray(np.asarray(bq, dtype=np.float32)),
        "bk": np.ascontiguousarray(np.asarray(bk, dtype=np.float32)),
        "bv": np.ascontiguousarray(np.asarray(bv, dtype=np.float32) * s),
    }
    return [
        {"x": np.ascontiguousarray(xt[c * NB : (c + 1) * NB]), **in_common}
        for c in range(NCORES)
    ]


def kernel(x, Wq, bq, Wk, bk, Wv, bv):
    nc = _get_nc()
    in_maps = prepare_in_maps(x, Wq, bq, Wk, bk, Wv, bv)
    res = run_bass_kernel_spmd(nc, in_maps, list(range(NCORES))).results
    return np.concatenate([r["out"] for r in res], axis=0)


# revision 16
# speedup vs baseline: 2.0390x; 1.0006x over previous
"""Multi-head attention (B=64, N=577, E=768, H=8) on 8 Trainium2 NeuronCores.

Sharding: data-parallel over batch — each core gets 8 batches, full weights.

Per-core dataflow (per batch):
  x [577,768] --load--> SBUF, PE-transpose -> xT [768(6x128), 577]
  QT = Wq.T @ xT  (head-packed [96, 8*578]), KT likewise        (PE, fp16)
  V  = xT.T @ Wv + bv, scaled by 1/sqrt(D), ones-column per head (-> rowsums)
  per head h:
    eT[k,q] = KT_h.T @ QT_h   (k on partitions, 5 k-tiles)      (PE, fp16)
    expE = exp(eT)            (no max-subtraction; |e| <~ 19)   (ACT, ->f32r)
    aT[d,q] (+rowsum row) = sum_k V_h[k,d+1] * expE[k,q]        (PE, f32r)
    attn[q,d] = transpose(aT) / rowsum                          (PE + DVE)
  out[b] assembled in SBUF [128,5,768], DMA'd back.

softmax(e)*scaling @ (x Wv + bv) == (exp(e) @ (s*(x Wv + bv))) / rowsum(exp(e)),
so the host passes Wv*s, bv*s (as fp16) and the kernel never multiplies by s.

Scheduling: the attention phase is rate-limited by ACT's exp (~740ns per
[128,578] tile) — PE produces an energy tile in ~540ns, so a naive ordering
stalls PE on PSUM slots every k-tile and the HAM clock gate drops the PE to
1.2 GHz. To keep PE streak-busy, batch b+1's x-load/transpose and QKV
projection matmuls are interleaved as filler work between batch b's
energy/AV matmuls (generator `p01_units` pumped inside the head loop).

fp32r matmul ISA restrictions: moving-operand and psum-dst innermost counts
must be even, dst 8B-aligned, start_partition 0 — hence the N->578 padded
free dim (pad column zeroed) and the even projection splits.
"""

import os
import numpy as np
from contextlib import ExitStack

import concourse.bass as bass
import concourse.bacc as bacc
import concourse.tile as tile
from concourse import mybir, masks
from concourse.bass_utils import run_bass_kernel_spmd


F32 = mybir.dt.float32
F32R = mybir.dt.float32r
F16 = mybir.dt.float16
BF16 = mybir.dt.bfloat16
Exp = mybir.ActivationFunctionType.Exp

B, N, E, H = 64, 577, 768, 8
D = E // H  # 96
NCORES = 8
NB = B // NCORES  # batches per core
NP = 578  # N padded to even (fp32r matmul moving/dst counts must be even)
NT = (N + 127) // 128  # 5 row tiles
ROWS = [min(128, N - t * 128) for t in range(NT)]  # [128,128,128,128,65]
EC = E // 128  # 6 contraction chunks
QSPLITS = [(0, 512), (512, NP)]  # 2-bank psum column splits (energy / AV)
PSPLITS = [(0, 290), (290, NP)]  # 1-bank psum splits for Q/K projections
VSPLITS = [(0, 384), (384, E)]  # 1-bank psum splits for V projection
DA = D + 1  # V columns per head incl. ones column
QCH = [(t * 128, min(128, NP - t * 128)) for t in range(NT)]  # q-chunks of 128


def build(nb=NB):
    nc = bacc.Bacc("TRN2", num_devices=NCORES)
    x_d = nc.declare_dram_parameter("x", [nb, E, N], F16, isOutput=False).ap()
    wq_d = nc.declare_dram_parameter("wq", [E, E], F16, isOutput=False).ap()
    wk_d = nc.declare_dram_parameter("wk", [E, E], F16, isOutput=False).ap()
    wv_d = nc.declare_dram_parameter("wv", [E, E], F16, isOutput=False).ap()
    bq_d = nc.declare_dram_parameter("bq", [E], F32, isOutput=False).ap()
    bk_d = nc.declare_dram_parameter("bk", [E], F32, isOutput=False).ap()
    bv_d = nc.declare_dram_parameter("bv", [E], F32, isOutput=False).ap()
    o_d = nc.declare_dram_parameter("out", [nb, N, E], F32, isOutput=True).ap()

    with tile.TileContext(nc) as tc:
        with ExitStack() as ctx:
            cpool = ctx.enter_context(tc.tile_pool(name="cpool", bufs=1))
            xt_pool = ctx.enter_context(tc.tile_pool(name="xt_pool", bufs=2))
            qt_pool = ctx.enter_context(tc.tile_pool(name="qt_pool", bufs=2))
            v_pool = ctx.enter_context(tc.tile_pool(name="v_pool", bufs=2))
            ee_pool = ctx.enter_context(tc.tile_pool(name="ee_pool", bufs=15))
            r_pool = ctx.enter_context(tc.tile_pool(name="r_pool", bufs=2))
            out_pool = ctx.enter_context(tc.tile_pool(name="out_pool", bufs=2))
            # "big" = 2-bank tiles (energy, AV); "small" = 1-bank tiles
            # (x-transpose blocks, attn-transpose blocks, projection halves)
            psb_pool = ctx.enter_context(
                tc.tile_pool(name="psb_pool", bufs=2, space="PSUM")
            )
            pss_pool = ctx.enter_context(
                tc.tile_pool(name="pss_pool", bufs=4, space="PSUM")
            )

            # ---- constants ----
            ident = cpool.tile([128, 128], F32)
            masks.make_identity(nc, ident[:, :])
            zero1 = cpool.tile([128, 1], F32)
            nc.gpsimd.memset(zero1[:, :], 0.0)
            one1 = cpool.tile([128, 1], F32)
            nc.gpsimd.memset(one1[:, :], 1.0)

            wq_sb = cpool.tile([128, EC * E], F16)
            wk_sb = cpool.tile([128, EC * E], F16)
            wv_sb = cpool.tile([128, EC * E], F16)
            bq_sb = cpool.tile([128, EC], F32)
            bk_sb = cpool.tile([128, EC], F32)
            bv_sb = cpool.tile([128, E], F32)

            def dma_weights():
                # One DMA instruction per weight matrix (issue cost on SyncE
                # is ~650ns per dma_start — chunked loads serialize startup).
                # Q/K weights first (first projection needs them); V last.
                nc.sync.dma_start(
                    out=wq_sb.rearrange("p (c e) -> p c e", c=EC),
                    in_=wq_d.rearrange("(c p) e -> p c e", p=128),
                )
                nc.sync.dma_start(
                    out=bq_sb[:, :], in_=bq_d.rearrange("(c p) -> p c", p=128)
                )
                nc.sync.dma_start(
                    out=wk_sb.rearrange("p (c e) -> p c e", c=EC),
                    in_=wk_d.rearrange("(c p) e -> p c e", p=128),
                )
                nc.sync.dma_start(
                    out=bk_sb[:, :], in_=bk_d.rearrange("(c p) -> p c", p=128)
                )
                nc.sync.dma_start(out=bv_sb[:, :], in_=bv_d.partition_broadcast(128))
                nc.sync.dma_start(
                    out=wv_sb.rearrange("p (c e) -> p c e", c=EC),
                    in_=wv_d.rearrange("(c p) e -> p c e", p=128),
                )

            def p01_units(b, first=False):
                """Load/transpose x and project Q/K/V for batch b.

                Generator yielding after each small chunk of PE work
                (~3-6 matmuls) so it can be pumped as filler between the
                previous batch's attention matmuls. The final yield carries
                the produced (qt, kt, vaug) tiles.
                """
                xt = xt_pool.tile([128, EC * NP], F16, tag="xt")
                xt6 = xt.rearrange("p (c n) -> p c n", c=EC)
                nc.vector.tensor_copy(
                    xt6[:, :, N:NP],
                    zero1[:, 0:1].unsqueeze(1).broadcast_to([128, EC, NP - N]),
                )
                nc.sync.dma_start(
                    out=xt6[:, :, 0:N],
                    in_=x_d[b].rearrange("(c p) n -> p c n", p=128),
                )
                if first:
                    dma_weights()

                # Dense M=128 projections into [128, 6, NP]; heads 0/3/4/7
                # are contiguous within one chunk (base partition 0 or 32),
                # heads 1/2/5/6 span a chunk boundary and are repacked into a
                # shadow [96, 4, NP] tile via SBUF->SBUF shift DMAs.
                qtd = qt_pool.tile([128, EC, NP], F16, tag="qtd")
                ktd = qt_pool.tile([128, EC, NP], F16, tag="ktd")
                qts = qt_pool.tile([D, 4, NP], F16, tag="qts")
                kts = qt_pool.tile([D, 4, NP], F16, tag="kts")
                for dstd, dsts, w_sb, b_sb in (
                    (qtd, qts, wq_sb, bq_sb),
                    (ktd, kts, wk_sb, bk_sb),
                ):
                    for c in range(EC):
                        for q0, q1 in PSPLITS:
                            p_ps = pss_pool.tile([128, q1 - q0], F32, tag="pss")
                            for cc in range(EC):
                                nc.tensor.matmul(
                                    p_ps[:, :],
                                    w_sb[:, cc * E + c * 128 : cc * E + (c + 1) * 128],
                                    xt6[:, cc, q0:q1],
                                    start=(cc == 0),
                                    stop=(cc == EC - 1),
                                )
                            nc.vector.tensor_scalar_add(
                                dstd[:, c, q0:q1], p_ps[:, :], b_sb[:, c : c + 1]
                            )
                            yield None
                    # repack chunk-spanning heads 1,2,5,6 -> shadow slots
                    for m, h in enumerate((1, 2, 5, 6)):
                        r0 = (h * D) % 128
                        c0 = (h * D) // 128
                        w0 = 128 - r0  # rows in first chunk
                        nc.sync.dma_start(
                            out=dsts[0:w0, m, :], in_=dstd[r0:128, c0, :]
                        )
                        nc.sync.dma_start(
                            out=dsts[w0:D, m, :], in_=dstd[0 : D - w0, c0 + 1, :]
                        )
                        if m % 2 == 1:
                            yield None

                # V (pre-scaled by host): vaug [128, NT, H, D+1], col D = ones
                vaug = v_pool.tile([128, NT, H, DA], BF16, tag="vaug")
                nc.vector.tensor_copy(
                    vaug[:, :, :, D : D + 1],
                    one1[:, 0:1]
                    .unsqueeze(1)
                    .unsqueeze(1)
                    .broadcast_to([128, NT, H, 1]),
                )
                for t in range(NT):
                    nr = ROWS[t]
                    for vi, (e0, e1) in enumerate(VSPLITS):
                        v_ps = pss_pool.tile([nr, e1 - e0], F32, tag="pss")
                        for c in range(EC):
                            nc.tensor.matmul(
                                v_ps[:, :],
                                xt6[:, c, t * 128 : t * 128 + nr],
                                wv_sb[:, c * E + e0 : c * E + e1],
                                start=(c == 0),
                                stop=(c == EC - 1),
                            )
                        h0 = vi * (H // 2)
                        nc.vector.tensor_add(
                            vaug[:nr, t, h0 : h0 + H // 2, 0:D],
                            v_ps.rearrange("p (h d) -> p h d", h=H // 2),
                            bv_sb[:nr, e0:e1].rearrange("p (h d) -> p h d", h=H // 2),
                        )
                        yield None
                yield (qtd, ktd, qts, kts, vaug)

            def p2(b, tiles, filler, result_box):
                """Attention for batch b; pumps `filler` between energy tiles."""
                qtd, ktd, qts, kts, vaug = tiles
                out_sb = out_pool.tile([128, NT, E], F32, tag="out")
                ee = {}  # (h, j) -> expE tile
                av = {}  # h -> AV psum tile [128, NT, DA] (q on partitions)

                def pump(n=1):
                    for _ in range(n):
                        try:
                            r = next(filler)
                            if r is not None:
                                result_box["tiles"] = r
                        except StopIteration:
                            return

                SHADOW = {1: 0, 2: 1, 5: 2, 6: 3}

                def qk_view(dense, shadow, h):
                    if h in SHADOW:
                        m = SHADOW[h]
                        return shadow[:, m, :]
                    r0 = (h * D) % 128
                    c0 = (h * D) // 128
                    return dense[r0 : r0 + D, c0, :]

                def emit_e(h, j):
                    nr = ROWS[j]
                    e_ps = psb_pool.tile([nr, NP], F32, tag="psb")
                    kv = qk_view(ktd, kts, h)
                    qv = qk_view(qtd, qts, h)
                    lhsT = kv[:, j * 128 : j * 128 + nr]
                    for q0, q1 in QSPLITS:
                        nc.tensor.matmul(e_ps[:, q0:q1], lhsT, qv[:, q0:q1])
                    t = ee_pool.tile([nr, NP], BF16, tag="ee", name="ee_t")
                    nc.scalar.activation(t[:, :], e_ps[:, :], Exp)
                    ee[(h, j)] = t

                def emit_av(h, t):
                    # attn[q, d] directly for q-chunk t: lhsT = expE chunk
                    # (stationary), rhs = V_h column block (97 cols incl
                    # ones). One complete accumulation chain per call — PSUM
                    # groups are bank-granular, so chains in the shared bank
                    # must not interleave.
                    if t == 0:
                        av[h] = pss_pool.tile(
                            [128, NT, DA], F32, tag="pss", name="av_ps"
                        )
                    c0, cw = QCH[t]
                    for j in range(NT):
                        nr = ROWS[j]
                        nc.tensor.matmul(
                            av[h][:cw, t, :],
                            ee[(h, j)][:, c0 : c0 + cw],
                            vaug[:nr, j, h, :],
                            start=(j == 0),
                            stop=(j == NT - 1),
                        )
                    if t == NT - 1:
                        for jj in range(NT):
                            del ee[(h, jj)]

                def emit_norm(h):
                    p = av.pop(h)
                    r = r_pool.tile([128, NT], F32, tag="r")
                    nc.vector.reciprocal(r[:, 0 : NT - 1], p[:, 0 : NT - 1, D])
                    nc.vector.reciprocal(
                        r[: ROWS[-1], NT - 1 : NT], p[: ROWS[-1], NT - 1, D : D + 1]
                    )
                    nc.vector.tensor_mul(
                        out_sb[:, 0 : NT - 1, h * D : (h + 1) * D],
                        p[:, 0 : NT - 1, 0:D],
                        r[:, 0 : NT - 1].unsqueeze(2).broadcast_to([128, NT - 1, D]),
                    )
                    nr = ROWS[-1]
                    nc.vector.tensor_mul(
                        out_sb[:nr, NT - 1, h * D : (h + 1) * D],
                        p[:nr, NT - 1, 0:D],
                        r[:nr, NT - 1 : NT].broadcast_to([nr, D]),
                    )

                for h in range(H):
                    for j in range(NT):
                        emit_e(h, j)
                        if h >= 2:
                            emit_av(h - 2, j)
                        pump(1)
                    if h >= 2:
                        emit_norm(h - 2)
                # tail: AV + norm for heads H-2, H-1
                for h in (H - 2, H - 1):
                    for j in range(NT):
                        emit_av(h, j)
                        pump(1)
                    emit_norm(h)
                pump(1000)  # drain any remaining filler

                for t in range(NT):
                    nr = ROWS[t]
                    nc.sync.dma_start(
                        out=o_d[b, t * 128 : t * 128 + nr, :], in_=out_sb[:nr, t, :]
                    )

            box0 = {}
            for r in p01_units(0, first=True):
                if r is not None:
                    box0["tiles"] = r
            tiles = box0["tiles"]
            for b in range(nb):
                box = {}
                filler = p01_units(b + 1) if b + 1 < nb else iter(())
                p2(b, tiles, filler, box)
                if b + 1 < nb:
                    tiles = box.get("tiles")
                    assert tiles is not None, "filler not fully pumped"
    nc.compile()
    return nc


_CACHE = {}


def _get_nc(nb=NB):
    if nb not in _CACHE:
        _CACHE[nb] = build(nb)
    return _CACHE[nb]


def prepare_in_maps(x, Wq, bq, Wk, bk, Wv, bv):
    s = float(D) ** -0.5
    # x enters the kernel pre-transposed ([b, E, N]) in fp16 — the kernel
    # rounds xT to fp16 anyway, so this is numerically identical and saves
    # all on-chip transposes.
    xt = np.ascontiguousarray(
        np.asarray(x, dtype=np.float32).transpose(0, 2, 1).astype(np.float16)
    )
    in_common = {
        "wq": np.ascontiguousarray(np.asarray(Wq, dtype=np.float16)),
        "wk": np.ascontiguousarray(np.asarray(Wk, dtype=np.float16)),
        "wv": np.ascontiguousarray(
            (np.asarray(Wv, dtype=np.float32) * s).astype(np.float16)
        ),
        "bq": np.ascontiguousarray(np.asarray(bq, dtype=np.float32)),
        "bk": np.ascontiguousarray(np.asarray(bk, dtype=np.float32)),
        "bv": np.ascontiguousarray(np.asarray(bv, dtype=np.float32) * s),
    }
    return [
        {"x": np.ascontiguousarray(xt[c * NB : (c + 1) * NB]), **in_common}
        for c in range(NCORES)
    ]


def kernel(x, Wq, bq, Wk, bk, Wv, bv):
    nc = _get_nc()
    in_maps = prepare_in_maps(x, Wq, bq, Wk, bk, Wv, bv)
    res = run_bass_kernel_spmd(nc, in_maps, list(range(NCORES))).results
    return np.concatenate([r["out"] for r in res], axis=0)
